# revision 10
# baseline (speedup 1.0000x reference)
"""Causal self-attention kernel for 8 TRN2 NeuronCores.

Problem (hardcoded): B=2, T=4096, C=768, NH=12, HS=64.
  qkv = x @ w_attn + b_attn; per-head causal softmax attention;
  y = att_out @ w_proj + b_proj

Sharding: 24 (batch, head) units over 8 cores -> 3 heads per core.
  cores 0..3: batch 0, heads (0,1,2), (3,4,5), (6,7,8), (9,10,11)
  cores 4..7: batch 1, same head split.
Each core computes a partial y^T [C, T] (its heads' contribution to the
output projection); the host sums partials per batch and adds b_proj.

On-chip dataflow per core (f32r matmuls, fp32 PSUM accumulation):
  phase 1: x tiles -> PE transpose -> x^T chunks; QKV^T = w_slice.T @ x^T
           giving Q^T/K^T in [head_dim, T] layout; V^T blocks are 65 rows
           (64 v columns + a ones row synthesized by a zero weight column
           with bias 1.0) transposed per 128-token tile into V' [128, 65].
  phase 2: per (q-block 512, k-super 1024): S^T = K^T_tile.T @ Q^T block,
           exp on ScalarE (scale=1/8), causal mask on diagonal tiles (DVE),
           O'^T += V'.T @ P^T accumulated in PSUM ([65, 512]: row 64 = softmax
           denominator via the ones column), then normalize via reciprocal +
           PE broadcast + DVE multiply.
  phase 3: y^T = sum_h wp_h.T @ O_norm_h accumulated over the 3 heads.
"""

import numpy as np

B, T, C, NH = 2, 4096, 768, 12
HS = C // NH          # 64
NCORES = 8
HPC = 3               # heads per core
QB = 512              # q block (moving dim)
NQB = T // QB         # 8
NKT = T // 128        # 32 k-tiles
NTB = T // QB         # t-blocks in phase 1
NCC = C // 128        # 6 contraction chunks
VP_W = 66             # V' width per k-tile (64 + ones + zero pad)
WQJ = 582             # wq columns: 128+128+64+64+66+66+66

_CACHE = {}


def _build():
    import contextlib
    import concourse.bacc as bacc
    import concourse.mybir as mybir
    from concourse.tile import TileContext
    from concourse.masks import make_identity

    f32 = mybir.dt.float32
    f32r = mybir.dt.float32r
    Exp = mybir.ActivationFunctionType.Exp
    mult = mybir.AluOpType.mult

    nc = bacc.Bacc(trn_type="TRN2")

    x = nc.dram_tensor("x", [T, C], f32, kind="ExternalInput")
    wq = nc.dram_tensor("wq", [C, WQJ], f32, kind="ExternalInput")
    bq = nc.dram_tensor("bq", [128, 7], f32, kind="ExternalInput")
    wp = nc.dram_tensor("wp", [192, C], f32, kind="ExternalInput")
    y = nc.dram_tensor("y", [C, T], f32, kind="ExternalOutput")

    # j-block layout of wq columns (assembled by host):
    #   0: [qA|qB] 128   1: [kA|kB] 128   2: qC 64   3: kC 64
    #   4: vA' 66        5: vB' 66        6: vC' 66
    #   (col 64 zero w/ bias 1 -> ones row; col 65 zero pad for even width)
    JBLK = [(0, 128), (128, 128), (256, 64), (320, 64),
            (384, 66), (450, 66), (516, 66)]

    with TileContext(nc) as tc, nc.allow_low_precision("f32r kernel"):
        with contextlib.ExitStack() as ctx:
            cpool = ctx.enter_context(tc.tile_pool(name="const", bufs=1))
            keep = ctx.enter_context(tc.tile_pool(name="keep", bufs=1))
            ident_f = cpool.tile([128, 128], f32)
            make_identity(nc, ident_f[:])
            ident = cpool.tile([128, 128], f32r)
            nc.vector.tensor_copy(ident[:], ident_f[:])
            # mask[k, jj] = 1.0 iff jj - k >= 384 ; diagonal k-tile at offset m
            # uses slice [:, 384-m : 896-m]
            mask = cpool.tile([128, 896], f32)
            nc.gpsimd.memset(mask[:], 1.0)
            nc.gpsimd.affine_select(
                out=mask[:], in_=mask[:], compare_op=mybir.AluOpType.is_ge,
                fill=0.0, base=-384, channel_multiplier=-1, pattern=[[1, 896]])
            ones_t = cpool.tile([128, 64], f32)
            nc.gpsimd.memset(ones_t[:], 1.0)
            ones_r = cpool.tile([128, 64], f32r)
            nc.vector.tensor_copy(ones_r[:], ones_t[:])

            wq_sb = cpool.tile([128, NCC, WQJ], f32r)
            nc.gpsimd.dma_start(wq_sb[:],
                                wq.rearrange("(cc p) j -> p cc j", p=128))
            bq_sb = cpool.tile([128, 7], f32)
            nc.sync.dma_start(bq_sb[:], bq[:, :])
            wp_sb = [keep.tile([64, C], f32r, tag=f"wp{h}", name=f"wp{h}")
                     for h in range(HPC)]
            for h in range(HPC):
                nc.gpsimd.dma_start(wp_sb[h][:], wp[h * 64:(h + 1) * 64, :])

            # persistent activations
            QT_AB = keep.tile([128, T], f32r, tag="qt_ab")
            KT_AB = keep.tile([128, T], f32r, tag="kt_ab")
            QT_C = keep.tile([64, T], f32r, tag="qt_c")
            KT_C = keep.tile([64, T], f32r, tag="kt_c")
            Vp = [keep.tile([128, NKT * VP_W], f32r, tag=f"vp{h}", name=f"vp{h}")
                  for h in range(HPC)]
            ON = [keep.tile([64, T], f32r, tag=f"on{h}", name=f"on{h}")
                  for h in range(HPC)]

            # ---------------- phase 1: x^T, QKV^T, V' ----------------
            with (
                tc.tile_pool(name="p1x", bufs=5) as p1x,
                tc.tile_pool(name="p1s", bufs=2) as p1s,
                tc.tile_pool(name="p1xt", bufs=1) as p1xt,
                tc.tile_pool(name="ps1", bufs=1, space="PSUM") as ps1,
            ):
                for tb in range(NTB):
                    xs = [p1x.tile([128, C], f32r, tag="xs", name=f"xs{i}")
                          for i in range(4)]
                    for i in range(4):
                        t0 = tb * QB + i * 128
                        nc.gpsimd.dma_start(xs[i][:], x[t0:t0 + 128, :])
                    xt = p1xt.tile([128, NCC, QB], f32r, tag="xt")
                    for cc in range(NCC):
                        xtp = ps1.tile([128, QB], f32r, tag=f"xtp{cc % 3}",
                                       name=f"xtp{cc}")
                        for i in range(4):
                            nc.tensor.transpose(
                                xtp[:, i * 128:(i + 1) * 128],
                                xs[i][:, cc * 128:(cc + 1) * 128], ident[:])
                        nc.vector.tensor_copy(xt[:, cc, :], xtp[:])

                    stage = [None] * 7
                    for blk in range(7):
                        j0, m = JBLK[blk]
                        qp = ps1.tile([128, QB], f32, tag=f"qkvp{blk % 2}",
                                      name=f"qp{blk}")
                        for cc in range(NCC):
                            nc.tensor.matmul(
                                qp[0:m, :], wq_sb[:, cc, j0:j0 + m],
                                xt[:, cc, :],
                                start=(cc == 0), stop=(cc == NCC - 1))
                        t0 = tb * QB
                        if blk == 0:
                            dest = QT_AB[:, t0:t0 + QB]
                        elif blk == 1:
                            dest = KT_AB[:, t0:t0 + QB]
                        elif blk == 2:
                            dest = QT_C[:, t0:t0 + QB]
                        elif blk == 3:
                            dest = KT_C[:, t0:t0 + QB]
                        else:
                            stage[blk] = p1s.tile([66, QB], f32r,
                                                  tag=f"stage{blk}",
                                                  name=f"stage{blk}")
                            dest = stage[blk][:]
                        nc.vector.tensor_scalar_add(
                            dest, qp[0:m, :], bq_sb[0:m, blk:blk + 1])

                    # V' build: transpose V'^T [65, 128] slices into Vp
                    for h in range(HPC):
                        src = stage[4 + h]
                        vtp = ps1.tile([128, 4, VP_W], f32r, tag=f"vtp{h % 2}",
                                       name=f"vtp{h}")
                        for i in range(4):
                            nc.tensor.transpose(
                                vtp[:, i, :],
                                src[:, i * 128:(i + 1) * 128],
                                ident[0:66, 0:66])
                        kt0 = tb * 4
                        vview = Vp[h][:].rearrange("p (kt w) -> p kt w", w=VP_W)
                        nc.vector.tensor_copy(vview[:, kt0:kt0 + 4, :], vtp[:])

            # ---------------- phase 2 + 3: attention, projection -------------
            with (
                tc.tile_pool(name="p2", bufs=3) as p2,
                tc.tile_pool(name="p2r", bufs=2) as p2r,
                tc.tile_pool(name="p2y", bufs=3) as p2y,
                tc.tile_pool(name="ps2", bufs=1, space="PSUM") as ps2,
            ):
                for qb in range(NQB):
                    q0 = qb * QB
                    nkt = 4 * qb + 4
                    for h in range(HPC):
                        if h < 2:
                            kt_t, qt_t, pp = KT_AB, QT_AB, 64 * h
                        else:
                            kt_t, qt_t, pp = KT_C, QT_C, 0
                        ov = ps2.tile([66, QB], f32, tag=f"ov{h % 2}",
                                      name=f"ov{h}")
                        for s in range(nkt // 2):
                            sp = ps2.tile([128, 1024], f32, tag=f"sps{s % 2}",
                                          name=f"sp{s}")
                            for j in range(2):
                                kt = 2 * s + j
                                nc.tensor.matmul(
                                    sp[:, j * QB:(j + 1) * QB],
                                    kt_t[pp:pp + 64, kt * 128:(kt + 1) * 128],
                                    qt_t[pp:pp + 64, q0:q0 + QB],
                                    start=True, stop=True)
                            pt = p2.tile([128, 1024], f32r, tag="pt")
                            nc.scalar.activation(pt[:], sp[:], Exp, scale=0.125)
                            for j in range(2):
                                kt = 2 * s + j
                                m = kt * 128 - q0
                                if 0 <= m < QB:  # diagonal tile: causal mask
                                    nc.vector.tensor_tensor(
                                        out=pt[:, j * QB:(j + 1) * QB],
                                        in0=pt[:, j * QB:(j + 1) * QB],
                                        in1=mask[:, 384 - m:896 - m], op=mult)
                            for j in range(2):
                                kt = 2 * s + j
                                nc.tensor.matmul(
                                    ov[:], Vp[h][:, kt * VP_W:(kt + 1) * VP_W],
                                    pt[:, j * QB:(j + 1) * QB],
                                    start=(s == 0 and j == 0),
                                    stop=(s == nkt // 2 - 1 and j == 1))
                        # normalize: recip of row 64, PE-broadcast, multiply
                        rr = p2r.tile([66, QB], f32r, tag="rr")
                        nc.vector.reciprocal(rr[64:65, :], ov[64:65, :])
                        rbp = ps2.tile([64, QB], f32, tag="rbp")
                        nc.tensor.matmul(rbp[:], ones_r[64:65, :], rr[64:65, :],
                                         start=True, stop=True)
                        rrb = p2r.tile([64, QB], f32, tag="rrb")
                        nc.vector.tensor_copy(rrb[:], rbp[:])
                        nc.vector.tensor_tensor(
                            out=ON[h][:, q0:q0 + QB], in0=ov[0:64, :],
                            in1=rrb[:], op=mult)
                    # phase 3 for this t-block: y^T[:, q0:q0+QB]
                    for co in range(NCC):
                        yp = ps2.tile([128, QB], f32, tag="yp")
                        for h in range(HPC):
                            nc.tensor.matmul(
                                yp[:], wp_sb[h][:, co * 128:(co + 1) * 128],
                                ON[h][:, q0:q0 + QB],
                                start=(h == 0), stop=(h == HPC - 1))
                        ys = p2y.tile([128, QB], f32, tag="ys")
                        nc.vector.tensor_copy(ys[:], yp[:])
                        nc.sync.dma_start(
                            y[co * 128:(co + 1) * 128, q0:q0 + QB], ys[:])

    nc.finalize()
    return nc


def _core_inputs(x, w_attn, b_attn, w_proj):
    """Build the 8 per-core input maps (numpy float32)."""
    maps = []
    z1 = np.zeros((C, 1), np.float32)
    for core in range(NCORES):
        b = core // 4
        heads = [HPC * (core % 4) + k for k in range(HPC)]
        hA, hB, hC = heads
        qc = lambda h: slice(h * HS, (h + 1) * HS)
        kc = lambda h: slice(C + h * HS, C + (h + 1) * HS)
        vc = lambda h: slice(2 * C + h * HS, 2 * C + (h + 1) * HS)
        wqm = np.concatenate([
            w_attn[:, qc(hA)], w_attn[:, qc(hB)],
            w_attn[:, kc(hA)], w_attn[:, kc(hB)],
            w_attn[:, qc(hC)], w_attn[:, kc(hC)],
            w_attn[:, vc(hA)], z1, z1, w_attn[:, vc(hB)], z1, z1,
            w_attn[:, vc(hC)], z1, z1,
        ], axis=1)
        bqm = np.zeros((128, 7), np.float32)
        bqm[0:64, 0] = b_attn[qc(hA)]
        bqm[64:128, 0] = b_attn[qc(hB)]
        bqm[0:64, 1] = b_attn[kc(hA)]
        bqm[64:128, 1] = b_attn[kc(hB)]
        bqm[0:64, 2] = b_attn[qc(hC)]
        bqm[0:64, 3] = b_attn[kc(hC)]
        for i, h in enumerate(heads):
            bqm[0:64, 4 + i] = b_attn[vc(h)]
            bqm[64, 4 + i] = 1.0
        wpm = np.concatenate([w_proj[h * HS:(h + 1) * HS, :] for h in heads],
                             axis=0)
        maps.append({
            "x": np.ascontiguousarray(x[b], np.float32),
            "wq": np.ascontiguousarray(wqm, np.float32),
            "bq": np.ascontiguousarray(bqm, np.float32),
            "wp": np.ascontiguousarray(wpm, np.float32),
        })
    return maps


def run_cores(in_maps, trace=False):
    from concourse import bass_utils
    if "nc" not in _CACHE:
        _CACHE["nc"] = _build()
    return bass_utils.run_bass_kernel_spmd(
        _CACHE["nc"], in_maps, list(range(NCORES)), trace=trace)


def kernel(x, w_attn, b_attn, w_proj, b_proj):
    x = np.asarray(x, np.float32)
    w_attn = np.asarray(w_attn, np.float32)
    b_attn = np.asarray(b_attn, np.float32)
    w_proj = np.asarray(w_proj, np.float32)
    b_proj = np.asarray(b_proj, np.float32)

    in_maps = _core_inputs(x, w_attn, b_attn, w_proj)
    res = run_cores(in_maps)
    y = np.zeros((B, T, C), np.float32)
    for b in range(B):
        acc = np.zeros((C, T), np.float64)
        for core in range(4 * b, 4 * b + 4):
            acc += res.results[core]["y"].astype(np.float64)
        y[b] = acc.T + b_proj[None, :]
    return y


# revision 12
# speedup vs baseline: 1.2466x; 1.2466x over previous
"""Causal self-attention kernel for 8 TRN2 NeuronCores.

Problem (hardcoded): B=2, T=4096, C=768, NH=12, HS=64.
  qkv = x @ w_attn + b_attn; per-head causal softmax attention;
  y = att_out @ w_proj + b_proj

Sharding: 24 (batch, head) units over 8 cores -> 3 heads per core.
  cores 0..3: batch 0, heads (0,1,2), (3,4,5), (6,7,8), (9,10,11)
  cores 4..7: batch 1, same head split.
Each core computes a partial y^T [C, T]; the host sums partials per batch
and adds b_proj. The host also pre-transposes x (x^T is pure input
marshalling), so the kernel loads x^T chunks directly.

Per-core dataflow (f32r matmuls, fp32 PSUM accumulation), interleaved so
the PE never idles across phase boundaries (keeps the HAM clock warm):
  for tb in 0..7:
    QKV^T(tb) = w_blocks.T @ x^T(tb): Q^T/K^T [head_dim, T] layout; head C's
      Q/K are written twice ([qC|qC] weight blocks) so its S matmuls can
      alternate PE row groups like A/B do. V'^T blocks are 66 rows (64 v +
      ones row from a zero weight column with bias 1.0 + zero pad row),
      PE-transposed per 128-token tile into V' [128, 66].
    attention(qb=tb): per k-tile: S^T_A and S^T_B computed into one
      [128, 1024] PSUM pair with ALTERNATING PE row groups (A rows 0-63,
      B rows 64-127 -> the hardware overlaps them); one wide exp on ScalarE
      (scale=1/8) PSUM->SBUF; causal mask on diagonal tiles (DVE);
      O'^T += V'.T @ P^T accumulated in PSUM [66, 512] (row 64 = softmax
      denominator); then reciprocal + PE broadcast + DVE normalize.
      Head C runs the same way using its duplicated-row Q/K tiles
      (even k-tiles on rows 0-63, odd on rows 64-127).
    projection(qb=tb): y^T = sum_h wp_h.T @ O_norm_h.
"""

import numpy as np

B, T, C, NH = 2, 4096, 768, 12
HS = C // NH          # 64
NCORES = 8
HPC = 3               # heads per core
QB = 512              # q block (moving dim)
NQB = T // QB         # 8
NKT = T // 128        # 32 k-tiles
NTB = T // QB         # t-blocks
NCC = C // 128        # 6 contraction chunks
VP_W = 66             # V' width per k-tile (64 + ones + zero pad)
WQJ = 4 * 128 + 3 * 66   # 710

_CACHE = {}


def _build():
    import contextlib
    import concourse.bacc as bacc
    import concourse.mybir as mybir
    from concourse.tile import TileContext
    from concourse.masks import make_identity

    f32 = mybir.dt.float32
    f32r = mybir.dt.float32r
    Exp = mybir.ActivationFunctionType.Exp
    mult = mybir.AluOpType.mult

    nc = bacc.Bacc(trn_type="TRN2")

    xt_d = nc.dram_tensor("xt", [C, T], f32, kind="ExternalInput")
    wq = nc.dram_tensor("wq", [C, WQJ], f32, kind="ExternalInput")
    bq = nc.dram_tensor("bq", [128, 7], f32, kind="ExternalInput")
    wp = nc.dram_tensor("wp", [192, C], f32, kind="ExternalInput")
    y = nc.dram_tensor("y", [C, T], f32, kind="ExternalOutput")

    # j-blocks: 0:[qA|qB] 1:[kA|kB] 2:[qC|qC] 3:[kC|kC] 4:vA' 5:vB' 6:vC'
    JBLK = [(0, 128), (128, 128), (256, 128), (384, 128),
            (512, 66), (578, 66), (644, 66)]

    with TileContext(nc) as tc, nc.allow_low_precision("f32r kernel"):
        with contextlib.ExitStack() as ctx:
            cpool = ctx.enter_context(tc.tile_pool(name="const", bufs=1))
            keep = ctx.enter_context(tc.tile_pool(name="keep", bufs=1))
            xtp_p = ctx.enter_context(tc.tile_pool(name="xtp", bufs=2))
            stg_p = ctx.enter_context(tc.tile_pool(name="stg", bufs=2))
            pt_p = ctx.enter_context(tc.tile_pool(name="ptp", bufs=3))
            on_p = ctx.enter_context(tc.tile_pool(name="onp", bufs=2))
            rr_p = ctx.enter_context(tc.tile_pool(name="rrp", bufs=2))
            ys_p = ctx.enter_context(tc.tile_pool(name="ysp", bufs=2))
            sps_p = ctx.enter_context(
                tc.tile_pool(name="sps", bufs=2, space="PSUM"))
            ov_p = ctx.enter_context(
                tc.tile_pool(name="ovp", bufs=1, space="PSUM"))
            sm_p = ctx.enter_context(
                tc.tile_pool(name="smp", bufs=2, space="PSUM"))

            ident_f = cpool.tile([128, 128], f32)
            make_identity(nc, ident_f[:])
            ident = cpool.tile([128, 128], f32r)
            nc.vector.tensor_copy(ident[:], ident_f[:])
            mask = cpool.tile([128, 896], f32)
            nc.gpsimd.memset(mask[:], 1.0)
            nc.gpsimd.affine_select(
                out=mask[:], in_=mask[:], compare_op=mybir.AluOpType.is_ge,
                fill=0.0, base=-384, channel_multiplier=-1, pattern=[[1, 896]])
            ones_t = cpool.tile([128, 64], f32)
            nc.gpsimd.memset(ones_t[:], 1.0)
            ones_r = cpool.tile([128, 64], f32r)
            nc.vector.tensor_copy(ones_r[:], ones_t[:])

            wq_sb = cpool.tile([128, NCC, WQJ], f32r)
            nc.gpsimd.dma_start(wq_sb[:],
                                wq.rearrange("(cc p) j -> p cc j", p=128))
            bq_sb = cpool.tile([128, 7], f32)
            nc.sync.dma_start(bq_sb[:], bq[:, :])
            wp_sb = [keep.tile([64, C], f32r, tag=f"wp{h}", name=f"wp{h}")
                     for h in range(HPC)]
            for h in range(HPC):
                nc.gpsimd.dma_start(wp_sb[h][:], wp[h * 64:(h + 1) * 64, :])

            QT_AB = keep.tile([128, T], f32r, tag="qt_ab")
            KT_AB = keep.tile([128, T], f32r, tag="kt_ab")
            QT_C = keep.tile([128, T], f32r, tag="qt_c")
            KT_C = keep.tile([128, T], f32r, tag="kt_c")
            Vp = [keep.tile([128, NKT * VP_W], f32r, tag=f"vp{h}",
                            name=f"vp{h}") for h in range(HPC)]

            def qkv_block(tb):
                t0 = tb * QB
                xt = xtp_p.tile([128, NCC, QB], f32r, tag="xt")
                for cc in range(NCC):
                    nc.gpsimd.dma_start(
                        xt[:, cc, :],
                        xt_d[cc * 128:(cc + 1) * 128, t0:t0 + QB])
                stage = [None] * 7
                for blk in range(7):
                    j0, m = JBLK[blk]
                    qp = sm_p.tile([128, QB], f32, tag="small",
                                   name=f"qp{tb}_{blk}")
                    for cc in range(NCC):
                        nc.tensor.matmul(
                            qp[0:m, :], wq_sb[:, cc, j0:j0 + m], xt[:, cc, :],
                            start=(cc == 0), stop=(cc == NCC - 1))
                    if blk == 0:
                        dest = QT_AB[:, t0:t0 + QB]
                    elif blk == 1:
                        dest = KT_AB[:, t0:t0 + QB]
                    elif blk == 2:
                        dest = QT_C[:, t0:t0 + QB]
                    elif blk == 3:
                        dest = KT_C[:, t0:t0 + QB]
                    else:
                        stage[blk] = stg_p.tile([66, QB], f32r,
                                                tag=f"stage{blk}",
                                                name=f"stage{tb}_{blk}")
                        dest = stage[blk][:]
                    nc.vector.tensor_scalar_add(
                        dest, qp[0:m, :], bq_sb[0:m, blk:blk + 1])
                for h in range(HPC):
                    src = stage[4 + h]
                    vtp = sm_p.tile([128, 4, VP_W], f32r, tag="small",
                                    name=f"vtp{tb}_{h}")
                    for i in range(4):
                        nc.tensor.transpose(
                            vtp[:, i, :], src[:, i * 128:(i + 1) * 128],
                            ident[0:VP_W, 0:VP_W])
                    kt0 = tb * 4
                    vview = Vp[h][:].rearrange("p (kt w) -> p kt w", w=VP_W)
                    nc.vector.tensor_copy(vview[:, kt0:kt0 + 4, :], vtp[:])

            def s_pair(qb, sps, half, kt, kt_t, qt_t, rows):
                """S^T for one head-half into sps[:, half*QB:...]."""
                q0 = qb * QB
                r0, r1 = rows
                nc.tensor.matmul(
                    sps[:, half * QB:(half + 1) * QB],
                    kt_t[r0:r1, kt * 128:(kt + 1) * 128],
                    qt_t[r0:r1, q0:q0 + QB], start=True, stop=True)

            def mask_and_av(qb, sps_pt, half, kt, ov, start, stop, h):
                q0 = qb * QB
                m = kt * 128 - q0
                if 0 <= m < QB:
                    nc.vector.tensor_tensor(
                        out=sps_pt[:, half * QB:(half + 1) * QB],
                        in0=sps_pt[:, half * QB:(half + 1) * QB],
                        in1=mask[:, 384 - m:896 - m], op=mult)
                nc.tensor.matmul(
                    ov[:], Vp[h][:, kt * VP_W:(kt + 1) * VP_W],
                    sps_pt[:, half * QB:(half + 1) * QB],
                    start=start, stop=stop)

            def normalize(qb, h, ov, dest):
                q0 = qb * QB
                rr = rr_p.tile([66, QB], f32r, tag="rr", name=f"rr{qb}_{h}")
                nc.vector.reciprocal(rr[64:65, :], ov[64:65, :])
                rbp = sm_p.tile([64, QB], f32, tag="small", name=f"rb{qb}_{h}")
                nc.tensor.matmul(rbp[:], ones_r[64:65, :], rr[64:65, :],
                                 start=True, stop=True)
                rrb = rr_p.tile([64, QB], f32, tag="rrb", name=f"rc{qb}_{h}")
                nc.vector.tensor_copy(rrb[:], rbp[:])
                nc.vector.tensor_tensor(out=dest, in0=ov[0:64, :],
                                        in1=rrb[:], op=mult)

            for tb in range(NTB):
                qkv_block(tb)
                qb = tb
                q0 = qb * QB
                nkt = 4 * qb + 4

                # heads A, B: row-group-alternated S, shared exp
                ovA = ov_p.tile([66, QB], f32, tag="ovA", name=f"ovA{qb}")
                ovB = ov_p.tile([66, QB], f32, tag="ovB", name=f"ovB{qb}")
                for kt in range(nkt):
                    sps = sps_p.tile([128, 1024], f32, tag="sps",
                                     name=f"sAB{qb}_{kt}")
                    s_pair(qb, sps, 0, kt, KT_AB, QT_AB, (0, 64))
                    s_pair(qb, sps, 1, kt, KT_AB, QT_AB, (64, 128))
                    pt = pt_p.tile([128, 1024], f32r, tag="pt")
                    nc.scalar.activation(pt[:], sps[:], Exp, scale=0.125)
                    mask_and_av(qb, pt, 0, kt, ovA, kt == 0, kt == nkt - 1, 0)
                    mask_and_av(qb, pt, 1, kt, ovB, kt == 0, kt == nkt - 1, 1)
                onA = on_p.tile([64, QB], f32r, tag="on0", name=f"onA{qb}")
                onB = on_p.tile([64, QB], f32r, tag="on1", name=f"onB{qb}")
                normalize(qb, 0, ovA, onA[:])
                normalize(qb, 1, ovB, onB[:])

                # head C: alternation via duplicated rows (even kt low,
                # odd kt high)
                ovC = ov_p.tile([66, QB], f32, tag="ovA", name=f"ovC{qb}")
                for s in range(nkt // 2):
                    kt0, kt1 = 2 * s, 2 * s + 1
                    sps = sps_p.tile([128, 1024], f32, tag="sps",
                                     name=f"sC{qb}_{s}")
                    s_pair(qb, sps, 0, kt0, KT_C, QT_C, (0, 64))
                    s_pair(qb, sps, 1, kt1, KT_C, QT_C, (64, 128))
                    pt = pt_p.tile([128, 1024], f32r, tag="pt")
                    nc.scalar.activation(pt[:], sps[:], Exp, scale=0.125)
                    mask_and_av(qb, pt, 0, kt0, ovC, s == 0, False, 2)
                    mask_and_av(qb, pt, 1, kt1, ovC, False,
                                s == nkt // 2 - 1, 2)
                onC = on_p.tile([64, QB], f32r, tag="on2", name=f"onC{qb}")
                normalize(qb, 2, ovC, onC[:])

                # projection for this q-block
                ons = [onA, onB, onC]
                for co in range(NCC):
                    yp = sm_p.tile([128, QB], f32, tag="small",
                                   name=f"yp{qb}_{co}")
                    for h in range(HPC):
                        nc.tensor.matmul(
                            yp[:], wp_sb[h][:, co * 128:(co + 1) * 128],
                            ons[h][:], start=(h == 0), stop=(h == HPC - 1))
                    ys = ys_p.tile([128, QB], f32, tag="ys",
                                   name=f"ys{qb}_{co}")
                    nc.vector.tensor_copy(ys[:], yp[:])
                    nc.sync.dma_start(
                        y[co * 128:(co + 1) * 128, q0:q0 + QB], ys[:])

    nc.finalize()
    return nc


def _core_inputs(x, w_attn, b_attn, w_proj):
    """Build the 8 per-core input maps (numpy float32)."""
    maps = []
    zc = np.zeros((C, 1), np.float32)
    for core in range(NCORES):
        b = core // 4
        heads = [HPC * (core % 4) + k for k in range(HPC)]
        hA, hB, hC = heads
        qc = lambda h: slice(h * HS, (h + 1) * HS)
        kc = lambda h: slice(C + h * HS, C + (h + 1) * HS)
        vc = lambda h: slice(2 * C + h * HS, 2 * C + (h + 1) * HS)
        wqm = np.concatenate([
            w_attn[:, qc(hA)], w_attn[:, qc(hB)],
            w_attn[:, kc(hA)], w_attn[:, kc(hB)],
            w_attn[:, qc(hC)], w_attn[:, qc(hC)],
            w_attn[:, kc(hC)], w_attn[:, kc(hC)],
            w_attn[:, vc(hA)], zc, zc, w_attn[:, vc(hB)], zc, zc,
            w_attn[:, vc(hC)], zc, zc,
        ], axis=1)
        bqm = np.zeros((128, 7), np.float32)
        bqm[0:64, 0] = b_attn[qc(hA)]
        bqm[64:128, 0] = b_attn[qc(hB)]
        bqm[0:64, 1] = b_attn[kc(hA)]
        bqm[64:128, 1] = b_attn[kc(hB)]
        bqm[0:64, 2] = b_attn[qc(hC)]
        bqm[64:128, 2] = b_attn[qc(hC)]
        bqm[0:64, 3] = b_attn[kc(hC)]
        bqm[64:128, 3] = b_attn[kc(hC)]
        for i, h in enumerate(heads):
            bqm[0:64, 4 + i] = b_attn[vc(h)]
            bqm[64, 4 + i] = 1.0
        wpm = np.concatenate([w_proj[h * HS:(h + 1) * HS, :] for h in heads],
                             axis=0)
        maps.append({
            "xt": np.ascontiguousarray(x[b].T, np.float32),
            "wq": np.ascontiguousarray(wqm, np.float32),
            "bq": np.ascontiguousarray(bqm, np.float32),
            "wp": np.ascontiguousarray(wpm, np.float32),
        })
    return maps


def run_cores(in_maps, trace=False):
    from concourse import bass_utils
    if "nc" not in _CACHE:
        _CACHE["nc"] = _build()
    return bass_utils.run_bass_kernel_spmd(
        _CACHE["nc"], in_maps, list(range(NCORES)), trace=trace)


def kernel(x, w_attn, b_attn, w_proj, b_proj):
    x = np.asarray(x, np.float32)
    w_attn = np.asarray(w_attn, np.float32)
    b_attn = np.asarray(b_attn, np.float32)
    w_proj = np.asarray(w_proj, np.float32)
    b_proj = np.asarray(b_proj, np.float32)

    in_maps = _core_inputs(x, w_attn, b_attn, w_proj)
    res = run_cores(in_maps)
    y = np.zeros((B, T, C), np.float32)
    for b in range(B):
        acc = np.zeros((C, T), np.float64)
        for core in range(4 * b, 4 * b + 4):
            acc += res.results[core]["y"].astype(np.float64)
        y[b] = acc.T + b_proj[None, :]
    return y
